# revision 1
# baseline (speedup 1.0000x reference)
"""Trainium2 Bass kernel for nn_ActorCritic_38886633898257.

Computes, for each batch row b of x (B, S, E):
  pairs[t]  = concat(x[b, t], x[b, t+1])            t in [0, S-2]
  h         = relu(pairs @ W1 + b1)
  scores[t] = h @ W2[:, 0]                          (+ b2, shift-invariant)
  logits    = scores masked to t < len_b - 1
  logp      = log_softmax(logits)
  out[b]    = (logp[action_b], entropy(logits))

Strategy: pure data parallel over 8 NeuronCores (32 rows each). Rows are
globally sorted by length and dealt round-robin so all cores see the same
per-slot padded length; per-slot lengths are compile-time constants, which
makes the whole program static while only paying ~3% padded work.

The host packs each core's x rows (only the first L_j positions of each
slot, split into two 128-feature planes) into one contiguous (rows, 128)
bf16 array. The kernel transpose-loads it with a few large xbar DMAs into
SBUF as xT (features on partitions, positions free) — big contiguous
transfers keep the xbar at full rate, and per-slot "x[t+1]" access is a
free +1 column shift. Then per slot:
  - mm1: 4 psum tiles (128 g x TL), each = 4 accumulated matmuls with
    stationary 128x128 bf16 W1 chunks (fast weight load)
  - relu(+b1) PSUM->SBUF, alternating ACT/DVE, output bf16
  - mm2: per-slot scores via 4-slot column-tiled groups (each slot's M=1
    matmuls land in their own 32-partition PE column group and run
    concurrently), staged + scattered into a (32, 512) scores matrix
Then one batched masked-softmax block (fp32) computes logp-at-action and
entropy for all 32 rows and DMAs out (32, 2).
"""

import numpy as np
import ml_dtypes
from contextlib import ExitStack

import concourse.bass as bass
import concourse.tile as tile
from concourse import mybir
from concourse.bass_utils import run_bass_kernel_spmd
import bass_rust

F32 = mybir.dt.float32
BF16 = mybir.dt.bfloat16
NP_BF16 = ml_dtypes.bfloat16
N_CORES = 8
B, S, E = 256, 512, 256
BC = B // N_CORES  # rows per core
NEG = -1e9
NCHUNK = 4         # transpose-load chunks (pipeline loads with compute)

# scheduling knobs (tuned empirically; see bench history)
KNOBS = {
    "nchunk": 8,      # transpose-load chunks
    "hps_bufs": 6,    # mm1 psum double-buffering (banks)
    "scps_bufs": 2,   # mm2 psum banks
    "stage_eng": "alt",  # 'alt' | 'act' | 'dve'
    "mm2_group": 4,   # slots per col-tiled mm2 batch (1..4)
    "zero_b1": True,  # b1 == 0 (per spec fill): fuse relu over g-pairs
    "h_bufs": 16,
}

AF = mybir.ActivationFunctionType
ALU = mybir.AluOpType
AX = mybir.AxisListType


def _nchunk():
    return KNOBS["nchunk"]


def _lp16(L):
    return min((L + 15) // 16 * 16, S)


def _layout(slot_len):
    """Column layout of the packed xT: per slot, [plane0 | plane1] blocks,
    grouped into NCHUNK chunk tiles at slot granularity."""
    sizes = [2 * _lp16(L) for L in slot_len]
    NCHUNK = _nchunk()
    tot = sum(sizes)
    # split slots into NCHUNK groups with roughly equal rows
    bounds = [0]
    acc, target, k = 0, tot / NCHUNK, 1
    for j, sz in enumerate(sizes):
        acc += sz
        if acc >= k * target and len(bounds) <= NCHUNK - 1:
            bounds.append(j + 1)
            k += 1
    while len(bounds) < NCHUNK + 1:
        bounds.append(BC)
    bounds[-1] = BC
    # per-slot (chunk_idx, col offset within chunk)
    slot_pos, chunk_rows = [], []
    for c in range(NCHUNK):
        off = 0
        for j in range(bounds[c], bounds[c + 1]):
            slot_pos.append((c, off))
            off += sizes[j]
        chunk_rows.append(off)
    return sizes, bounds, slot_pos, chunk_rows, tot


# --------------------------------------------------------------------------
# walrus in this toolchain rejects instructions with more than one sync wait
# ("Too many sync wait commands"); split extras onto preceding same-engine
# NOP carriers.
_MAXW = 1


def _split_sync_waits(nc):
    for bb in nc.main_func.blocks:
        il = bb.instructions
        i = 0
        while i < len(il):
            ins = il[i]
            si = ins.sync_info
            if si is not None and len(si.on_wait) > _MAXW:
                waits = list(si.on_wait)
                keep, rest = waits[-_MAXW:], waits[:-_MAXW]
                ins.sync_info = bass_rust.SyncInfo(
                    on_wait=keep, on_update=list(si.on_update))
                carriers = []
                for k in range(0, len(rest), _MAXW):
                    nop = mybir.InstNoOp(
                        name=f"waitsplit-{nc.next_id()}", ins=[], outs=[])
                    nop.engine = ins.engine
                    nop.sync_info = bass_rust.SyncInfo(
                        on_wait=rest[k:k + _MAXW], on_update=[])
                    carriers.append(nop)
                for j, nop in enumerate(carriers):
                    il.insert(i + j, nop)
                i += len(carriers)
            i += 1


# --------------------------------------------------------------------------
def _build_program(slot_len, repeat=1, probe=None):
    """Emit the SPMD program. slot_len: 32 compile-time padded lengths.

    repeat > 1 wraps the compute pipeline in a device-side loop (timing
    amplification only; results stay correct).
    probe: None | 'dma' | 'mm1' | 'relu' — truncate the pipeline after that
    stage (timing probes; output then undefined)."""
    nc = bass.Bass()
    stage_order = [None, 'relu', 'mm1', 'dma']
    lvl = stage_order.index(probe)  # 0 = full

    sizes, bounds, slot_pos, chunk_rows, tot = _layout(slot_len)

    x_d = nc.declare_dram_parameter("x", [tot, 128], BF16, isOutput=False)
    w1_d = nc.declare_dram_parameter("w1", [128, 16, 128], BF16, isOutput=False)
    w2_d = nc.declare_dram_parameter("w2", [128, 4], BF16, isOutput=False)
    b1_d = nc.declare_dram_parameter("b1", [128, 4], F32, isOutput=False)
    mb_d = nc.declare_dram_parameter("maskbias", [BC, S], F32, isOutput=False)
    oh_d = nc.declare_dram_parameter("onehot", [BC, S], F32, isOutput=False)
    out_d = nc.declare_dram_parameter("out", [BC, 2], F32, isOutput=True)

    with ExitStack() as ctx:
        tc = ctx.enter_context(tile.TileContext(nc))
        singles = ctx.enter_context(tc.tile_pool(name="singles", bufs=1))
        xt_bufs = 2 if tot <= 24576 else 1
        xt_p = ctx.enter_context(tc.tile_pool(name="xt", bufs=xt_bufs))
        hps_bufs = 3 if KNOBS["zero_b1"] else KNOBS["hps_bufs"]  # 2-bank tiles when paired
        hps_p = ctx.enter_context(tc.tile_pool(name="hps", bufs=hps_bufs, space="PSUM"))
        h_p = ctx.enter_context(tc.tile_pool(name="h", bufs=KNOBS["h_bufs"]))
        scps_p = ctx.enter_context(tc.tile_pool(name="scps", bufs=KNOBS["scps_bufs"], space="PSUM"))
        stage_p = ctx.enter_context(tc.tile_pool(name="stage", bufs=12))
        sm_p = ctx.enter_context(tc.tile_pool(name="sm", bufs=1))

        # --- one-time loads -------------------------------------------------
        w1_sb = singles.tile([128, 16, 128], BF16)
        nc.sync.dma_start(out=w1_sb, in_=w1_d[:, :, :])
        w2_sb = singles.tile([128, 4], BF16)
        nc.sync.dma_start(out=w2_sb, in_=w2_d[:, :])
        b1_sb = singles.tile([128, 4], F32)
        nc.sync.dma_start(out=b1_sb, in_=b1_d[:, :])
        mb_sb = singles.tile([BC, S], F32)
        nc.sync.dma_start(out=mb_sb, in_=mb_d[:, :])
        oh_sb = singles.tile([BC, S], F32)
        nc.sync.dma_start(out=oh_sb, in_=oh_d[:, :])

        # Pull the exp/ln activation tables in early so the ~2.7us table DMA
        # overlaps the main pipeline instead of landing in the tail.
        warm = singles.tile([1, 2], F32)
        nc.vector.memset(warm, 1.0)
        nc.scalar.activation(warm[:, 0:1], warm[:, 0:1], AF.Exp)
        nc.scalar.activation(warm[:, 1:2], warm[:, 1:2], AF.Ln)

        args = (slot_len, lvl, sizes, bounds, slot_pos, chunk_rows,
                x_d, w1_sb, w2_sb, b1_sb,
                xt_p, hps_p, h_p, scps_p, stage_p)
        if repeat > 1:
            scores_all = singles.tile([BC, S], F32, tag="sa")
            # one-time zero: scatters fully overwrite [0:TL_j] every
            # iteration and only the tail columns need the zeros, so a
            # per-iteration memset would just re-serialize on all 32
            # prior scatters at each iteration boundary
            nc.vector.memset(scores_all, 0.0)
            with tc.For_i(0, repeat, 1, hint_engines=(mybir.EngineType.PE,)):
                _emit_rep(nc, scores_all, *args)
            _emit_softmax(nc, sm_p, scores_all, mb_sb, oh_sb, out_d)
        else:
            scores_all = singles.tile([BC, S], F32, tag="sa")
            nc.vector.memset(scores_all, 0.0)
            _emit_rep(nc, scores_all, *args)
            _emit_softmax(nc, sm_p, scores_all, mb_sb, oh_sb, out_d)

    _split_sync_waits(nc)
    return nc




def _emit_mm2_batch(nc, batch, scps_p, stage_p, w2_sb, scores_all):
    """mm2 for up to 4 slots: one PSUM bank, each slot in its own
    32-partition column group -> the matmuls run concurrently on PE."""
    sc_ps = scps_p.tile([128, 512], F32, tag="scps")
    for g in range(4):
        for s, (js, TLs, hs) in enumerate(batch):
            nc.tensor.matmul(sc_ps[32 * s:32 * s + 1, 0:TLs],
                             w2_sb[:, g:g + 1], hs[g][:, 0:TLs],
                             start=(g == 0), stop=(g == 3),
                             tile_position=(0, 32 * s),
                             skip_group_check=True)
    for s, (js, TLs, hs) in enumerate(batch):
        stg = stage_p.tile([1, 512], F32, tag="stage")
        se = KNOBS["stage_eng"]
        if se == "act" or (se == "alt" and js % 2 == 0):
            nc.scalar.copy(out=stg[0:1, 0:TLs],
                           in_=sc_ps[32 * s:32 * s + 1, 0:TLs])
        else:
            nc.vector.tensor_copy(out=stg[0:1, 0:TLs],
                                  in_=sc_ps[32 * s:32 * s + 1, 0:TLs])
        nc.sync.dma_start(out=scores_all[js:js + 1, 0:TLs],
                          in_=stg[0:1, 0:TLs])

def _emit_rep(nc, scores_all, slot_len, lvl, sizes, bounds, slot_pos,
              chunk_rows, x_d, w1_sb, w2_sb, b1_sb,
              xt_p, hps_p, h_p, scps_p, stage_p):
    # chunked transpose loads: (rows, 128) DRAM -> (128, rows) SBUF bf16
    NCHUNK = _nchunk()
    xts = []
    roff = 0
    for c in range(NCHUNK):
        rows = chunk_rows[c]
        xt = xt_p.tile([128, rows], BF16, tag=f"xt{c}")
        nc.sync.dma_start_transpose(out=xt[:, 0:rows],
                                    in_=x_d[roff:roff + rows, :])
        xts.append(xt)
        roff += rows

    group = []
    pending = []
    for j in range(BC):
        L = int(slot_len[j])
        TL = L - 1
        Lp = _lp16(L)
        c, off = slot_pos[j]
        xt = xts[c]
        u0, u1 = off, off + Lp  # plane0 / plane1 column bases

        if lvl >= 3:
            stg = stage_p.tile([1, 512], F32, tag="stage")
            nc.vector.tensor_copy(stg[0:1, 0:4],
                                  xt[0:1, u0:u0 + 8].bitcast(F32))
            nc.sync.dma_start(out=scores_all[j:j + 1, 0:4],
                                in_=stg[0:1, 0:4])
            continue

        # mm1 per 128-wide g block; relu fused over g-pairs when b1 == 0
        hj = []
        zb = KNOBS["zero_b1"]
        hps = []
        for g in range(4):
            if zb:
                if g % 2 == 0:
                    hp2 = hps_p.tile([128, 2, 512], F32, tag="hps")
                hp = hp2[:, g % 2, :]
            else:
                hp = hps_p.tile([128, 512], F32, tag="hps1")
            hps.append(hp)
            # chunk index e*4+g, e in 0..3 (0,1: W1 top; 2,3: bottom)
            nc.tensor.matmul(hp[:, 0:TL], w1_sb[:, 0 * 4 + g, :],
                             xt[:, u0:u0 + TL], start=True, stop=False)
            nc.tensor.matmul(hp[:, 0:TL], w1_sb[:, 1 * 4 + g, :],
                             xt[:, u1:u1 + TL], start=False, stop=False)
            nc.tensor.matmul(hp[:, 0:TL], w1_sb[:, 2 * 4 + g, :],
                             xt[:, u0 + 1:u0 + L], start=False, stop=False)
            nc.tensor.matmul(hp[:, 0:TL], w1_sb[:, 3 * 4 + g, :],
                             xt[:, u1 + 1:u1 + L], start=False, stop=True)
            if lvl >= 2:
                stg = stage_p.tile([1, 512], F32, tag="stage")
                nc.vector.tensor_copy(stg[0:1, 0:4], hp[0:1, 0:4])
                nc.sync.dma_start(out=scores_all[j:j + 1, 4 * g:4 * g + 4],
                                  in_=stg[0:1, 0:4])
                continue

            if zb:
                # relu two psum banks in one op once the pair is complete
                if g % 2 == 1:
                    h2 = h_p.tile([128, 2, 512], BF16, tag="h")
                    if g == 1:
                        nc.scalar.activation(h2[:, :, 0:TL], hp2[:, :, 0:TL],
                                             AF.Relu)
                    else:
                        nc.vector.tensor_scalar_max(h2[:, :, 0:TL],
                                                    hp2[:, :, 0:TL], 0.0)
                    hj.append(h2[:, 0, :])
                    hj.append(h2[:, 1, :])
            else:
                h = h_p.tile([128, 512], BF16, tag="h1")
                if g % 2 == 0:
                    nc.scalar.activation(h[:, 0:TL], hp[:, 0:TL], AF.Relu,
                                         bias=b1_sb[:, g:g + 1], scale=1.0)
                else:
                    nc.vector.tensor_scalar(h[:, 0:TL], hp[:, 0:TL],
                                            b1_sb[:, g:g + 1], 0.0,
                                            op0=ALU.add, op1=ALU.max)
                hj.append(h)
            if lvl >= 1 and len(hj) > g:
                nc.sync.dma_start(out=scores_all[j:j + 1, 4 * g:4 * g + 1],
                                  in_=hj[g][0:1, 0:2].bitcast(F32))
        if lvl >= 1:
            continue
        group.append((j, TL, hj))

        # defer each full group's mm2 batch until after the NEXT slot's mm1
        # so the PE has dense work while the group's last relus complete
        if pending:
            _emit_mm2_batch(nc, pending, scps_p, stage_p, w2_sb, scores_all)
            pending.clear()
        if len(group) == KNOBS["mm2_group"]:
            pending = list(group)
            group.clear()

    if lvl == 0:
        if pending:
            _emit_mm2_batch(nc, pending, scps_p, stage_p, w2_sb, scores_all)
        if group:
            _emit_mm2_batch(nc, group, scps_p, stage_p, w2_sb, scores_all)


def _emit_softmax(nc, sm_p, scores_all, mb_sb, oh_sb, out_d):
    logits = sm_p.tile([BC, S], F32)
    nc.vector.tensor_add(logits, scores_all, mb_sb)
    rowmax = sm_p.tile([BC, 1], F32)
    nc.vector.reduce_max(rowmax, logits, axis=AX.X)
    zt = sm_p.tile([BC, S], F32)
    nc.vector.tensor_scalar_sub(zt, logits, rowmax)
    et = sm_p.tile([BC, S], F32)
    sumexp = sm_p.tile([BC, 1], F32)
    nc.scalar.activation(et, zt, AF.Exp, accum_out=sumexp)
    logsum = sm_p.tile([BC, 1], F32)
    nc.scalar.activation(logsum, sumexp, AF.Ln)
    rinv = sm_p.tile([BC, 1], F32)
    nc.vector.reciprocal(rinv, sumexp)
    logp = sm_p.tile([BC, S], F32)
    nc.vector.tensor_scalar_sub(logp, zt, logsum)

    scr0 = sm_p.tile([BC, S], F32)
    lp = sm_p.tile([BC, 1], F32)
    nc.vector.tensor_mul(scr0, logp, oh_sb)
    nc.vector.reduce_sum(lp, scr0, axis=AX.X)
    scr1 = sm_p.tile([BC, S], F32)
    ez = sm_p.tile([BC, 1], F32)
    nc.vector.tensor_mul(scr1, et, zt)
    nc.vector.reduce_sum(ez, scr1, axis=AX.X)
    # entropy = logsum - (sum e*z) / sumexp
    ent = sm_p.tile([BC, 1], F32)
    nc.vector.tensor_mul(ent, ez, rinv)
    nc.vector.tensor_sub(ent, logsum, ent)

    res = sm_p.tile([BC, 2], F32)
    nc.vector.tensor_copy(res[:, 0:1], lp)
    nc.vector.tensor_copy(res[:, 1:2], ent)
    nc.sync.dma_start(out=out_d[:, :], in_=res)


# --------------------------------------------------------------------------
def prepare(x, W1, b1, W2, b2, lengths, position_action):
    """Host-side sharding: returns (slot_len, in_maps, core_rows)."""
    x = np.asarray(x, np.float32)
    W1 = np.asarray(W1, np.float32)
    b1 = np.asarray(b1, np.float32)
    W2 = np.asarray(W2, np.float32)
    lengths = np.asarray(lengths)
    position_action = np.asarray(position_action)

    # length-sorted round-robin assignment: rank r -> core r%8, slot r//8
    order = np.argsort(lengths, kind="stable")
    slot_len = [int(lengths[order[j * N_CORES + N_CORES - 1]])
                for j in range(BC)]
    sizes, bounds, slot_pos, chunk_rows, tot = _layout(slot_len)

    # replicated params, pre-chunked for the 128x128 stationary loads
    w1c = np.ascontiguousarray(
        W1.reshape(4, 128, 4, 128).transpose(1, 0, 2, 3)
        .reshape(128, 16, 128)).astype(NP_BF16)
    w2c = np.ascontiguousarray(W2[:, 0].reshape(4, 128).T).astype(NP_BF16)
    b1c = np.ascontiguousarray(b1.reshape(4, 128).T)

    xb = x.astype(NP_BF16)  # (B, S, E)

    tcol = np.arange(S, dtype=np.int64)[None, :]
    in_maps, core_rows = [], []
    for core in range(N_CORES):
        rows = order[np.arange(BC) * N_CORES + core]
        core_rows.append(rows)
        # pack [plane0[:Lp] ; plane1[:Lp]] per slot, contiguously
        xp = np.empty((tot, 128), NP_BF16)
        pos = 0
        for j in range(BC):
            Lp = _lp16(slot_len[j])
            r = rows[j]
            xp[pos:pos + Lp] = xb[r, 0:Lp, 0:128]
            xp[pos + Lp:pos + 2 * Lp] = xb[r, 0:Lp, 128:256]
            pos += 2 * Lp
        assert pos == tot

        lens = lengths[rows].astype(np.int64)
        mb = np.where(tcol < (lens - 1)[:, None],
                      np.float32(0), np.float32(NEG)).astype(np.float32)
        oh = np.zeros((BC, S), np.float32)
        oh[np.arange(BC), position_action[rows].astype(np.int64)] = 1.0
        in_maps.append({
            "x": xp,
            "w1": w1c, "w2": w2c, "b1": b1c,
            "maskbias": mb, "onehot": oh,
        })
    return slot_len, in_maps, core_rows


_prog_cache = {}
LAST_RESULT = None


def kernel(x, W1, b1, W2, b2, lengths, position_action):
    slot_len, in_maps, core_rows = prepare(
        x, W1, b1, W2, b2, lengths, position_action)

    KNOBS["zero_b1"] = bool(np.all(np.asarray(b1) == 0))
    key = (tuple(slot_len), KNOBS["zero_b1"])
    if key not in _prog_cache:
        _prog_cache[key] = _build_program(slot_len)
    nc = _prog_cache[key]

    br = run_bass_kernel_spmd(nc, in_maps, list(range(N_CORES)))
    global LAST_RESULT
    LAST_RESULT = br

    out = np.zeros((B, 2), np.float32)
    for core in range(N_CORES):
        out[core_rows[core]] = br.results[core]["out"]
    return out



# revision 65
# speedup vs baseline: 1.7026x; 1.7026x over previous
"""Trainium2 Bass kernel for nn_ActorCritic_38886633898257.

Computes, for each batch row b of x (B, S, E):
  pairs[t]  = concat(x[b, t], x[b, t+1])            t in [0, S-2]
  h         = relu(pairs @ W1 + b1)
  scores[t] = h @ W2[:, 0]                          (+ b2, shift-invariant)
  logits    = scores masked to t < len_b - 1
  logp      = log_softmax(logits)
  out[b]    = (logp[action_b], entropy(logits))

Strategy: pure data parallel over 8 NeuronCores (32 rows each), rows
length-sorted and dealt round-robin so all cores share one compile-time
per-slot padded length (padded to %4 for the scatter layout).

All 32 slots' positions are concatenated into one dense column axis
(C = sum slot_len). x is packed AND transposed host-side as fp8e4
(128 feature partitions x 2 planes x C positions), so loads are plain
strided DMAs. mm1 runs as dense 512-column chunks with fp8 DoubleRow
matmuls (contraction 256 per pass, 2 fp8 MACs/cell/cycle): per chunk 4
psum blocks x 2 matmuls, where the "x_{t+1}" pass is the same operand
shifted one position. Chunks are processed in pairs with matmuls ordered
A(c0) A(c1) B(c0) B(c1) so each stationary weight load serves two
matmuls. W1 is scaled by 2^7 host-side to sit in fp8e4 normal range;
1/2^7 is folded into W2. Boundary columns between slots compute garbage
scores that land in mask-covered cells.

Per chunk: relu psum->SBUF bf16 (ACT/DVE alternating, 2 banks per op),
then mm2 (scores = h @ W2) as M=1 bf16 matmuls column-tiled 4 ways so 4
chunks' score rows accumulate concurrently in one psum bank. Scores are
staged (one psum->SBUF copy per 4-chunk group) and scattered with ONE
DMA per (slot x chunk) segment into a mod-4 interleaved quartered
layout: partition 32q+j holds slot j's positions t==q (mod 4) at column
t//4 (source AP (q,c) de-interleave <-> partition-strided dest).

The masked softmax then runs on all 128 partitions: logits=scores+mask,
exp with accumulated sumexp (scores are O(1) and masked cells -1e9, so
no max subtraction: exp underflows to exact 0), fused multiply-sum ops
for sum(et*logits) and the onehot dot, then one PE matmul against a 0/1
selection matrix to combine the four quarter partition groups, giving
logp[action] and entropy.
"""

import numpy as np
import ml_dtypes
from contextlib import ExitStack

import concourse.bass as bass
import concourse.tile as tile
from concourse import mybir
from concourse.bass_utils import run_bass_kernel_spmd
import bass_rust

F32 = mybir.dt.float32
BF16 = mybir.dt.bfloat16
FP8 = mybir.dt.float8e4
NP_BF16 = ml_dtypes.bfloat16
NP_FP8 = ml_dtypes.float8_e4m3
N_CORES = 8
B, S, E = 256, 512, 256
BC = B // N_CORES  # rows per core
NEG = -1e9
WSCALE = 128.0     # W1 fp8 scale (power of 2; folded out of W2)
CW = 512           # mm1/mm2 chunk width (psum bank = 512 f32)
OVL = 8            # x piece overlap columns (B-matmul reads col w+1)

KNOBS = {
    "mm2_group": 4,    # chunks per col-tiled mm2 psum bank (1..4)
    "hps_bufs": 3,     # mm1 psum tiles in flight (2 banks each)
    "scps_bufs": 2,    # mm2 psum banks
    "h_bufs": 10,      # relu output tiles (2 g-blocks each)
    "piece_chunks": 2, # compute chunks per x load piece
    "pair": True,      # chunk-paired matmul order (stationary reuse)
    "stage_eng": "alt",  # 'alt' | 'act' | 'dve' for group stage copies
    "scat_eng": "alt",   # engine issuing scatter DMAs: 'sp'|'act'|'alt'
    "zero_b1": True,
}

AF = mybir.ActivationFunctionType
ALU = mybir.AluOpType
AX = mybir.AxisListType
PM = mybir.MatmulPerfMode

PROBES = [None, 'sm', 'stage', 'mm2', 'relu', 'mm1', 'dma']


def _layout(slot_len):
    """Slot-aligned chunks: first-fit-decreasing pack whole slots into
    chunks of <= CW columns, so every slot's scores are contiguous in one
    chunk (scatter = one DMA per slot).

    Returns (widths, starts, slot_pos) where slot_pos[j] = (chunk, off)."""
    order = sorted(range(len(slot_len)), key=lambda j: -slot_len[j])
    bins = []       # list of [width, [(j, off), ...]]
    for j in order:
        L = int(slot_len[j])
        for b in bins:
            if b[0] + L <= CW:
                b[1].append((j, b[0]))
                b[0] += L
                break
        else:
            bins.append([L, [(j, 0)]])
    widths = [b[0] for b in bins]
    starts = np.concatenate([[0], np.cumsum(widths)]).astype(int)
    slot_pos = {}
    for c, b in enumerate(bins):
        for (j, off) in b[1]:
            slot_pos[j] = (c, off)
    return widths, starts, slot_pos


# --------------------------------------------------------------------------
# walrus in this toolchain rejects instructions with more than one sync wait
# ("Too many sync wait commands"); split extras onto preceding same-engine
# NOP carriers.
_MAXW = 1


def _split_sync_waits(nc):
    for bb in nc.main_func.blocks:
        il = bb.instructions
        i = 0
        while i < len(il):
            ins = il[i]
            si = ins.sync_info
            if si is not None and len(si.on_wait) > _MAXW:
                waits = list(si.on_wait)
                keep, rest = waits[-_MAXW:], waits[:-_MAXW]
                ins.sync_info = bass_rust.SyncInfo(
                    on_wait=keep, on_update=list(si.on_update))
                carriers = []
                for k in range(0, len(rest), _MAXW):
                    nop = mybir.InstNoOp(
                        name=f"waitsplit-{nc.next_id()}", ins=[], outs=[])
                    nop.engine = ins.engine
                    nop.sync_info = bass_rust.SyncInfo(
                        on_wait=rest[k:k + _MAXW], on_update=[])
                    carriers.append(nop)
                for j, nop in enumerate(carriers):
                    il.insert(i + j, nop)
                i += len(carriers)
            i += 1


# --------------------------------------------------------------------------
def _build_program(slot_len, repeat=1, probe=None):
    """Emit the SPMD program. slot_len: 32 compile-time padded lengths."""
    nc = bass.Bass()
    lvl = PROBES.index(probe)  # 0 = full

    widths, starts, slot_pos = _layout(slot_len)
    nch = len(widths)
    Cp = int(starts[-1]) + OVL   # packed cols incl pad

    x_d = nc.declare_dram_parameter("x", [128, 2, Cp], FP8, isOutput=False)
    w1_d = nc.declare_dram_parameter("w1", [128, 4, 2, 2, 128], FP8,
                                     isOutput=False)
    w2_d = nc.declare_dram_parameter("w2", [128, 4], BF16, isOutput=False)
    b1_d = nc.declare_dram_parameter("b1", [128, 4], F32, isOutput=False)
    mb_d = nc.declare_dram_parameter("maskbias", [BC, S], F32,
                                     isOutput=False)
    oh_d = nc.declare_dram_parameter("onehot", [BC, S], F32,
                                     isOutput=False)
    out_d = nc.declare_dram_parameter("out", [BC, 2], F32, isOutput=True)

    with ExitStack() as ctx:
        tc = ctx.enter_context(tile.TileContext(nc))
        singles = ctx.enter_context(tc.tile_pool(name="singles", bufs=1))
        xt_p = ctx.enter_context(tc.tile_pool(name="xt", bufs=1))
        hps_p = ctx.enter_context(
            tc.tile_pool(name="hps", bufs=KNOBS["hps_bufs"], space="PSUM"))
        h_p = ctx.enter_context(tc.tile_pool(name="h", bufs=KNOBS["h_bufs"]))
        scps_p = ctx.enter_context(
            tc.tile_pool(name="scps", bufs=KNOBS["scps_bufs"], space="PSUM"))
        stage_p = ctx.enter_context(tc.tile_pool(name="stage", bufs=4))
        sm_p = ctx.enter_context(tc.tile_pool(name="sm", bufs=1))

        # --- one-time loads ------------------------------------------------
        w1_sb = singles.tile([128, 4, 2, 2, 128], FP8)
        nc.sync.dma_start(out=w1_sb, in_=w1_d[:, :, :, :, :])
        w2_sb = singles.tile([128, 4], BF16)
        nc.sync.dma_start(out=w2_sb, in_=w2_d[:, :])
        b1_sb = singles.tile([128, 4], F32)
        nc.sync.dma_start(out=b1_sb, in_=b1_d[:, :])
        mb_sb = singles.tile([BC, S], F32)
        nc.sync.dma_start(out=mb_sb, in_=mb_d[:, :])
        oh_sb = singles.tile([BC, S], F32)
        nc.sync.dma_start(out=oh_sb, in_=oh_d[:, :])

        # pull the exp/ln activation tables in early so the table DMA
        # overlaps the main pipeline instead of landing in the tail
        warm = singles.tile([1, 2], F32)
        nc.vector.memset(warm, 1.0)
        nc.scalar.activation(warm[:, 0:1], warm[:, 0:1], AF.Exp)
        nc.scalar.activation(warm[:, 1:2], warm[:, 1:2], AF.Ln)

        scores4 = singles.tile([BC, S], F32, tag="sc4")
        # one-time zero: scatters fully overwrite the same cols every
        # iteration; never-written tail cols are masked by mb anyway
        nc.vector.memset(scores4, 0.0)

        args = (slot_len, widths, starts, slot_pos, lvl,
                x_d, w1_sb, w2_sb, b1_sb,
                xt_p, hps_p, h_p, scps_p, stage_p, scores4)
        if repeat > 1:
            # software-pipelined: the in-loop softmax consumes the PREVIOUS
            # iteration's scores (their scatters have long drained). Its ops
            # are interleaved between the first chunk pairs' emissions so
            # they fill ACT/DVE idle gaps instead of head-of-line-blocking
            # the relus; an epilogue softmax handles the final iteration.
            with tc.For_i(0, repeat, 1, hint_engines=(mybir.EngineType.PE,)):
                sm_steps = (_softmax_steps(nc, sm_p, scores4, mb_sb, oh_sb,
                                           out_d) if lvl == 0 else None)
                _emit_rep(nc, *args, sm_steps=sm_steps)
            if lvl == 0:
                _emit_softmax(nc, sm_p, scores4, mb_sb, oh_sb, out_d)
        else:
            _emit_rep(nc, *args)
            if lvl == 0:
                _emit_softmax(nc, sm_p, scores4, mb_sb, oh_sb, out_d)

    _split_sync_waits(nc)
    return nc


def _emit_mm1(nc, cs, xts, hps_p, h_p, w1_sb, b1_sb, lvl, widths, starts,
              stage_p, scores4):
    """mm1 + relu for a pair of chunks (or a single trailing chunk).

    Returns {c: [h_tile_gp0, h_tile_gp1]} (empty if probed out)."""
    PC = KNOBS["piece_chunks"]
    zb = KNOBS["zero_b1"]
    hjs = {c: [] for c in cs}
    for gp in range(2):
        hps = {}
        for c in cs:
            hps[c] = hps_p.tile([128, 2, CW], F32, tag="hps",
                                name=f"hps{c}g{gp}")
        for gi in range(2):
            g = 2 * gp + gi
            for e in range(2):
                for c in cs:
                    w = widths[c]
                    xt = xts[c // PC]
                    off = int(starts[c] - starts[(c // PC) * PC])
                    nc.tensor.matmul(
                        hps[c][:, gi, 0:w], w1_sb[:, g, e, :, :],
                        xt[:, :, off + e:off + w + e],
                        start=(e == 0), stop=(e == 1),
                        perf_mode=PM.DoubleRow)
        if lvl >= 5:  # probe: stop after mm1
            for c in cs:
                stg = stage_p.tile([1, 16], F32, tag="p")
                nc.vector.tensor_copy(stg[0:1, 0:4], hps[c][0:1, gp, 0:4])
                nc.sync.dma_start(out=scores4[0:1, 8 * gp:8 * gp + 4],
                                  in_=stg[0:1, 0:4])
            continue
        for ci, c in enumerate(cs):
            w = widths[c]
            h2 = h_p.tile([128, 2, CW], BF16, tag="h")
            if zb:
                if (gp + ci) % 2 == 0:
                    nc.scalar.activation(h2[:, :, 0:w], hps[c][:, :, 0:w],
                                         AF.Relu)
                else:
                    nc.vector.tensor_scalar_max(h2[:, :, 0:w],
                                                hps[c][:, :, 0:w], 0.0)
            else:
                for gi in range(2):
                    g = 2 * gp + gi
                    if (gp + ci) % 2 == 0:
                        nc.scalar.activation(h2[:, gi, 0:w],
                                             hps[c][:, gi, 0:w], AF.Relu,
                                             bias=b1_sb[:, g:g + 1],
                                             scale=1.0)
                    else:
                        nc.vector.tensor_scalar(h2[:, gi, 0:w],
                                                hps[c][:, gi, 0:w],
                                                b1_sb[:, g:g + 1], 0.0,
                                                op0=ALU.add, op1=ALU.max)
            hjs[c].append(h2)
    if lvl >= 4:  # probe: stop after relu (emit a consumer of h)
        for c in cs:
            for h2 in hjs[c]:
                nc.sync.dma_start(out=scores4[2:3, 0:2],
                                  in_=h2[0:1, 0:1, 0:4].bitcast(F32))
        return {}
    return hjs


def _emit_rep(nc, slot_len, widths, starts, slot_pos, lvl,
              x_d, w1_sb, w2_sb, b1_sb,
              xt_p, hps_p, h_p, scps_p, stage_p, scores4, sm_steps=None):
    nch = len(widths)
    GS = KNOBS["mm2_group"]
    PC = KNOBS["piece_chunks"]
    sm_steps = list(sm_steps or [])

    # per-piece loads (PC chunks each, +OVL overlap)
    xts = []
    for k in range((nch + PC - 1) // PC):
        c0 = k * PC
        w = sum(widths[c0:c0 + PC])
        p0 = int(starts[c0])
        w4 = (w + OVL + 3) // 4 * 4   # %4 strides keep bitcast probes legal
        xt = xt_p.tile([128, 2, w4], FP8, tag=f"xt{k}")
        nc.sync.dma_start(out=xt[:, :, 0:w + OVL],
                          in_=x_d[:, :, p0:p0 + w + OVL])
        xts.append(xt)

    if lvl >= 6:  # probe: loads only
        stg = stage_p.tile([1, 16], F32, tag="p")
        nc.vector.tensor_copy(stg[0:1, 0:4],
                              xts[-1][0:1, 0:1, 0:16].bitcast(F32))
        nc.sync.dma_start(out=scores4[0:1, 0:4], in_=stg[0:1, 0:4])
        return

    # scatter: each slot is whole inside one chunk
    scat = [[] for _ in range(nch)]
    for j in range(len(slot_len)):
        c, off = slot_pos[j]
        scat[c].append((off, off + int(slot_len[j]), j, 0))

    def issue_dma(i, out, in_):
        se = KNOBS["scat_eng"]
        eng = nc.scalar if (se == "act" or (se == "alt" and i % 2)) \
            else nc.sync
        eng.dma_start(out=out, in_=in_)

    group = []
    step = 2 if KNOBS["pair"] else 1
    for k0 in range(0, nch, step):
        cs = list(range(k0, min(k0 + step, nch)))
        hjs = _emit_mm1(nc, cs, xts, hps_p, h_p, w1_sb, b1_sb, lvl,
                        widths, starts, stage_p, scores4)
        if sm_steps:
            sm_steps.pop(0)()
        if lvl >= 4:
            continue
        for c in cs:
            group.append((c, hjs[c]))

        if len(group) < GS and cs[-1] != nch - 1:
            continue

        # mm2: interleave the group's accumulation chains across 4 column
        # groups so they run concurrently on the PE
        sc_ps = scps_p.tile([128, CW], F32, tag="scps")
        for g in range(4):
            for ci, (c, hj) in enumerate(group):
                w = widths[c]
                nc.tensor.matmul(sc_ps[32 * ci:32 * ci + 1, 0:w],
                                 w2_sb[:, g:g + 1], hj[g // 2][:, g % 2, 0:w],
                                 start=(g == 0), stop=(g == 3),
                                 tile_position=(0, 32 * ci),
                                 skip_group_check=True)
        if lvl >= 3:   # probe: stop after mm2
            stg = stage_p.tile([1, 16], F32, tag="p")
            nc.vector.tensor_copy(stg[0:1, 0:4], sc_ps[0:1, 0:4])
            nc.sync.dma_start(out=scores4[1:2, 0:4], in_=stg[0:1, 0:4])
            group = []
            continue

        # stage the group's score rows psum->SBUF in one copy
        np_ = 32 * (len(group) - 1) + 1
        gi = group[0][0] // GS
        stg = stage_p.tile([128, CW], F32, tag="stage")
        se = KNOBS["stage_eng"]
        if se == "act" or (se == "alt" and gi % 2 == 0):
            nc.scalar.copy(out=stg[0:np_, :], in_=sc_ps[0:np_, :])
        else:
            nc.vector.tensor_copy(out=stg[0:np_, :], in_=sc_ps[0:np_, :])
        if lvl >= 2:   # probe: stop after stage copy
            nc.sync.dma_start(out=scores4[3:4, 0:4], in_=stg[0:1, 4:8])
            group = []
            continue

        # scatter each slot piece straight from the stage tile into its
        # scores row (contiguous 1-partition copies)
        ndma = 0
        for ci, (c, _hj) in enumerate(group):
            sc0 = c * CW
            for (sa, sb, j, t0) in scat[c]:
                issue_dma(ndma, scores4[j:j + 1, t0:t0 + (sb - sa)],
                          stg[32 * ci:32 * ci + 1, sa:sb])
                ndma += 1
        group = []


def _softmax_steps(nc, sm_p, scores4, mb_sb, oh_sb, out_d):
    """Masked softmax on (BC, S) rows, returned as a list of emission
    thunks so the caller can interleave them with other work.

    Scores are O(1) and masked cols are -1e9, so exp needs no max
    subtraction (exp(-1e9) underflows to exactly 0)."""
    logits = sm_p.tile([BC, S], F32)
    et = sm_p.tile([BC, S], F32)
    junk = sm_p.tile([BC, S], F32)
    cmbt = sm_p.tile([BC, 3], F32)
    sc1 = sm_p.tile([BC, 4], F32)
    res = sm_p.tile([BC, 2], F32)
    logsum, rinv = sc1[:, 0:1], sc1[:, 1:2]

    def s_add():
        nc.vector.tensor_add(logits, scores4, mb_sb)

    def s_exp():
        nc.scalar.activation(et, logits, AF.Exp, accum_out=cmbt[:, 0:1])

    def s_s1():
        nc.vector.scalar_tensor_tensor(
            out=junk, in0=et, scalar=1.0, in1=logits,
            op0=ALU.mult, op1=ALU.mult, accum_out=cmbt[:, 1:2])

    def s_oh():
        nc.vector.scalar_tensor_tensor(
            out=junk, in0=logits, scalar=1.0, in1=oh_sb,
            op0=ALU.mult, op1=ALU.mult, accum_out=cmbt[:, 2:3])

    def s_fin():
        nc.scalar.activation(logsum, cmbt[:, 0:1], AF.Ln)
        nc.vector.reciprocal(rinv, cmbt[:, 0:1])
        # lp = score[action] - logsum
        nc.vector.tensor_sub(res[:, 0:1], cmbt[:, 2:3], logsum)
        # ent = logsum - S1/se
        nc.vector.tensor_mul(res[:, 1:2], cmbt[:, 1:2], rinv)
        nc.vector.tensor_sub(res[:, 1:2], logsum, res[:, 1:2])
        nc.sync.dma_start(out=out_d[:, :], in_=res)

    return [s_add, s_exp, s_s1, s_oh, s_fin]


def _emit_softmax(nc, sm_p, scores4, mb_sb, oh_sb, out_d):
    for step in _softmax_steps(nc, sm_p, scores4, mb_sb, oh_sb, out_d):
        step()


# --------------------------------------------------------------------------
def prepare(x, W1, b1, W2, b2, lengths, position_action):
    """Host-side sharding: returns (slot_len, in_maps, core_rows)."""
    x = np.asarray(x, np.float32)
    W1 = np.asarray(W1, np.float32)
    b1 = np.asarray(b1, np.float32)
    W2 = np.asarray(W2, np.float32)
    lengths = np.asarray(lengths)
    position_action = np.asarray(position_action)

    # length-sorted round-robin assignment: rank r -> core r%8, slot r//8
    order = np.argsort(lengths, kind="stable")
    slot_len = [int(lengths[order[j * N_CORES + N_CORES - 1]])
                for j in range(BC)]
    widths, starts, slot_pos = _layout(slot_len)
    Cp = int(starts[-1]) + OVL

    # W1 interleaved for DoubleRow: w1c[p, g, e, i, m]
    #   e=0: rows [0:256) (x_t), e=1: rows [256:512) (x_{t+1}); i = k-tile
    w1s = np.clip(W1 * WSCALE, -240.0, 240.0).astype(NP_FP8)
    w1c = np.empty((128, 4, 2, 2, 128), NP_FP8)
    for g in range(4):
        for e in range(2):
            for i in range(2):
                w1c[:, g, e, i, :] = w1s[256 * e + 128 * i:
                                         256 * e + 128 * i + 128,
                                         128 * g:128 * g + 128]
    w2c = np.ascontiguousarray(
        (W2[:, 0] / WSCALE).reshape(4, 128).T).astype(NP_BF16)
    b1c = np.ascontiguousarray((b1 * WSCALE).reshape(4, 128).T
                               .astype(np.float32))

    x8 = x.astype(NP_FP8)  # (B, S, E)

    in_maps, core_rows = [], []
    for core in range(N_CORES):
        rows = order[np.arange(BC) * N_CORES + core]
        core_rows.append(rows)
        xp = np.zeros((2, Cp, 128), NP_FP8)
        for j in range(BC):
            L = min(int(lengths[rows[j]]), slot_len[j])
            c, off = slot_pos[j]
            g0 = int(starts[c]) + off
            xp[0, g0:g0 + L] = x8[rows[j], 0:L, 0:128]
            xp[1, g0:g0 + L] = x8[rows[j], 0:L, 128:256]
        xt = np.ascontiguousarray(xp.transpose(2, 0, 1))  # (128, 2, Cp)

        lens = lengths[rows].astype(np.int64)
        acts = position_action[rows].astype(np.int64)
        tcol = np.arange(S, dtype=np.int64)[None, :]
        mb = np.where(tcol < (lens - 1)[:, None],
                      np.float32(0), np.float32(NEG)).astype(np.float32)
        oh = np.zeros((BC, S), np.float32)
        oh[np.arange(BC), acts] = 1.0
        in_maps.append({
            "x": xt,
            "w1": w1c, "w2": w2c, "b1": b1c,
            "maskbias": mb, "onehot": oh,
        })
    return slot_len, in_maps, core_rows


_prog_cache = {}
LAST_RESULT = None


def kernel(x, W1, b1, W2, b2, lengths, position_action):
    slot_len, in_maps, core_rows = prepare(
        x, W1, b1, W2, b2, lengths, position_action)

    KNOBS["zero_b1"] = bool(np.all(np.asarray(b1) == 0))
    key = (tuple(slot_len), KNOBS["zero_b1"])
    if key not in _prog_cache:
        _prog_cache[key] = _build_program(slot_len)
    nc = _prog_cache[key]

    br = run_bass_kernel_spmd(nc, in_maps, list(range(N_CORES)))
    global LAST_RESULT
    LAST_RESULT = br

    out = np.zeros((B, 2), np.float32)
    for core in range(N_CORES):
        out[core_rows[core]] = br.results[core]["out"]
    return out
